# revision 10
# baseline (speedup 1.0000x reference)
"""Trainium2 Bass kernel for nn_DeltaBlock: LN -> spatial edge conv -> residual
-> LN -> l2norm'd 2-layer MLP (gelu) -> residual.

Sharding: data-parallel over batch, 16 images / 8 cores = 2 images per core.
All heavy branch compute in bf16 channel-partition layout; the fp32 residual
path stays in token layout (branch outputs are scaled by gamma=1e-4, so bf16
there contributes ~4e-7 relative error on the output).

Math folds (exact, given norm2_w=1/norm2_b=0 which setup_inputs fixes):
  - l2norm(proto_in/out) columns normalized on host (bf16 weights).
  - channel-stage row l2norm of LN2's output: ||LN2(x2)|| = sqrt(DIM*var2)/
    sqrt(var2+eps) * ... folded exactly into the per-token scale
    S2' = (scale_in/sqrt(DIM)) * rsqrt(var2) applied to (x2 - m2).
  - gamma*scale_out folded into the per-channel e2 scale; gamma*alpha and
    gamma*(1-alpha)/HW folded into the e1 activation scale/bias.
"""
import numpy as np
import concourse.bass as bass
import concourse.bacc as bacc
import concourse.mybir as mybir
import concourse.tile as tile
from concourse import bass_utils

f32 = mybir.dt.float32
bf16 = mybir.dt.bfloat16
AF = mybir.ActivationFunctionType
ALU = mybir.AluOpType

DIM = 512
HID = 1024
EPS_LN = 1e-5
NCORES = 8
P = 128
NCT = DIM // P          # 4 channel tiles
NHT = HID // P          # 8 hidden tiles


def build_nc(b_local, h, w, s2_scale, num_devices=NCORES, reps=1):
    t_img = h * w
    n_chunks = max(1, t_img // 512)
    tc_sz = t_img // n_chunks
    assert tc_sz <= 512 and tc_sz % P == 0
    ntt_img = t_img // P
    ntt_chunk = tc_sz // P
    hp_h, hp_w = h + 2, w + 2

    nc = bacc.Bacc("TRN2", debug=False, num_devices=num_devices)

    x_d = nc.dram_tensor("x", [b_local * t_img, DIM], f32, kind="ExternalInput")
    w_in_d = nc.dram_tensor("w_in", [NCT, P, HID], bf16, kind="ExternalInput")
    w_out_d = nc.dram_tensor("w_out", [NHT, P, DIM], bf16, kind="ExternalInput")
    ga_d = nc.dram_tensor("ga", [NCT, P], f32, kind="ExternalInput")
    gb_d = nc.dram_tensor("gb", [NCT, P], f32, kind="ExternalInput")
    gs_d = nc.dram_tensor("gs", [NCT, P], f32, kind="ExternalInput")
    y_d = nc.dram_tensor("y", [b_local * t_img, DIM], f32, kind="ExternalOutput")

    with tile.TileContext(nc) as tc:
        with (
            tc.tile_pool(name="const", bufs=1) as cpool,
            tc.tile_pool(name="big", bufs=1) as big,
            tc.tile_pool(name="wk", bufs=2) as wk,
            tc.tile_pool(name="psum", bufs=1, space="PSUM") as pp,
        ):
            # ---- weights / constants ----
            w_in_sb = cpool.tile([P, NCT, HID], bf16, tag="w_in")
            for ct in range(NCT):
                nc.sync.dma_start(w_in_sb[:, ct, :], w_in_d[ct])
            w_out_sb = cpool.tile([P, NHT, DIM], bf16, tag="w_out")
            for ht in range(NHT):
                nc.sync.dma_start(w_out_sb[:, ht, :], w_out_d[ht])
            ga_sb = cpool.tile([P, NCT], f32, tag="ga")
            gb_sb = cpool.tile([P, NCT], f32, tag="gb")
            gs_sb = cpool.tile([P, NCT], f32, tag="gs")
            for v_sb, v_d in ((ga_sb, ga_d), (gb_sb, gb_d), (gs_sb, gs_d)):
                for ct in range(NCT):
                    nc.sync.dma_start(v_sb[:, ct:ct + 1],
                                      v_d[ct].rearrange("(p o) -> p o", o=1))
            ones_m = cpool.tile([P, P], bf16, tag="ones_m")
            nc.vector.memset(ones_m[:], 1.0 / DIM)
            ones_q = cpool.tile([P, P], bf16, tag="ones_q")
            nc.vector.memset(ones_q[:], 1.0)
            epsb = cpool.tile([P, 1], f32, tag="epsb")
            nc.vector.memset(epsb[:], EPS_LN)
            zerob = cpool.tile([P, 1], f32, tag="zerob")
            nc.vector.memset(zerob[:], 0.0)

            for img in range(b_local * reps):
                img = img % b_local
                tok0 = img * t_img

                # ---- x -> bf16 -> channel layout ----
                x_ch = big.tile([P, NCT, t_img], bf16, tag="x_ch")
                for tt in range(ntt_img):
                    xft = wk.tile([P, DIM], f32, tag="xft")
                    nc.sync.dma_start(
                        xft[:], x_d[tok0 + tt * P: tok0 + (tt + 1) * P, :])
                    xbt = wk.tile([P, DIM], bf16, tag="xbt")
                    nc.vector.tensor_copy(xbt[:], xft[:])
                    nc.scalar.dma_start_transpose(
                        x_ch[:, :, tt * P:(tt + 1) * P], xbt[:])

                # ---- LN1 stats ----
                s1_img = big.tile([P, t_img], bf16, tag="s1_img")
                t1_img = big.tile([P, t_img], bf16, tag="t1_img")
                for ch in range(n_chunks):
                    sl = slice(ch * tc_sz, (ch + 1) * tc_sz)
                    xsq = wk.tile([P, NCT, tc_sz], bf16, tag="xsq")
                    for ct in range(NCT):
                        nc.scalar.activation(xsq[:, ct, :], x_ch[:, ct, sl],
                                             AF.Square)
                    mps = pp.tile([P, 2, 512], f32, tag="stat")
                    m_b = mps[:, 0, :tc_sz]
                    ss_b = mps[:, 1, :tc_sz]
                    for ct in range(NCT):
                        nc.tensor.matmul(m_b, ones_m[:], x_ch[:, ct, sl],
                                         start=(ct == 0), stop=(ct == NCT - 1))
                    for ct in range(NCT):
                        nc.tensor.matmul(ss_b, ones_m[:], xsq[:, ct, :],
                                         start=(ct == 0), stop=(ct == NCT - 1))
                    m2 = wk.tile([P, tc_sz], f32, tag="m2")
                    nc.scalar.activation(m2[:], m_b, AF.Square)
                    var = wk.tile([P, tc_sz], f32, tag="var")
                    nc.vector.tensor_tensor(var[:], ss_b, m2[:], ALU.subtract)
                    nc.scalar.activation(s1_img[:, sl], var[:],
                                         AF.Abs_reciprocal_sqrt, bias=epsb[:])
                    nc.vector.tensor_tensor(t1_img[:, sl], s1_img[:, sl], m_b,
                                            ALU.mult)

                # ---- LN1 apply + spatial + e1 + x2, per channel tile ----
                e1_ch = big.tile([P, NCT, t_img], bf16, tag="e1_ch")
                x2_ch = x_ch
                s1_v = s1_img.rearrange("p (a b) -> p a b", b=w)
                t1_v = t1_img.rearrange("p (a b) -> p a b", b=w)
                for ct in range(NCT):
                    # padded h tile: zero borders, write interior = x*s - t
                    hp = wk.tile([P, hp_h, hp_w], bf16, tag="hp", bufs=1)
                    nc.vector.memset(hp[:, 0, :], 0.0)
                    nc.vector.memset(hp[:, hp_h - 1, :], 0.0)
                    nc.vector.memset(hp[:, 1:hp_h - 1, 0:1], 0.0)
                    nc.vector.memset(hp[:, 1:hp_h - 1, hp_w - 1:hp_w], 0.0)
                    gp_acc = wk.tile([P, 1], f32, tag="gp_acc", bufs=1)
                    x_v = x_ch[:, ct, :].rearrange("p (a b) -> p a b", b=w)
                    hin = hp[:, 1:h + 1, 1:w + 1]
                    nc.vector.scalar_tensor_tensor(
                        hin, x_v, 0.0, s1_v, ALU.bypass, ALU.mult)
                    nc.vector.scalar_tensor_tensor(
                        hin, hin, 0.0, t1_v, ALU.bypass, ALU.subtract,
                        accum_out=gp_acc[:])
                    e1b = wk.tile([P, 1], f32, tag="e1b", bufs=1)
                    nc.vector.tensor_tensor(e1b[:], gp_acc[:],
                                            gb_sb[:, ct:ct + 1], ALU.mult)
                    # vertical diffs |dv| over padded rows; feat = |dv|r + |dv|r+1
                    hpf = hp.rearrange("p a b -> p (a b)")
                    dv = wk.tile([P, (hp_h - 1) * hp_w], bf16, tag="dv", bufs=1)
                    feat = wk.tile([P, h, hp_w], bf16, tag="feat", bufs=1)
                    nc.vector.tensor_tensor(
                        dv[:], hpf[:, :(hp_h - 1) * hp_w], hpf[:, hp_w:],
                        ALU.subtract)
                    nc.vector.scalar_tensor_tensor(
                        dv[:], dv[:], -1.0, dv[:], ALU.mult, ALU.max)
                    featf = feat.rearrange("p a b -> p (a b)")
                    nc.vector.tensor_tensor(
                        featf[:], dv[:, :h * hp_w], dv[:, hp_w:], ALU.add)
                    # horizontal diffs on interior rows
                    dh = wk.tile([P, h, hp_w - 1], bf16, tag="dv", bufs=1)
                    nc.vector.tensor_tensor(
                        dh[:], hp[:, 1:h + 1, :hp_w - 1], hp[:, 1:h + 1, 1:],
                        ALU.subtract)
                    nc.vector.scalar_tensor_tensor(
                        dh[:], dh[:], -1.0, dh[:], ALU.mult, ALU.max)
                    nc.vector.tensor_tensor(
                        feat[:, :, 1:], feat[:, :, 1:], dh[:], ALU.add)
                    nc.vector.tensor_tensor(
                        feat[:, :, :hp_w - 1], feat[:, :, :hp_w - 1], dh[:],
                        ALU.add)
                    # e1 = ga*feat + gb*gp ; x2 = x + e1
                    nc.scalar.activation(
                        e1_ch[:, ct, :].rearrange("p (a b) -> p a b", b=w),
                        feat[:, :, 1:w + 1], AF.Identity,
                        bias=e1b[:], scale=ga_sb[:, ct:ct + 1])
                    nc.vector.tensor_tensor(
                        x2_ch[:, ct, :], x_ch[:, ct, :], e1_ch[:, ct, :],
                        ALU.add)

                # ---- LN2 stats: S2' = Abs_rsqrt(var2*s2_scale), T2' = S2'*m2 ----
                s2_img = s1_img
                t2_img = t1_img
                for ch in range(n_chunks):
                    sl = slice(ch * tc_sz, (ch + 1) * tc_sz)
                    xsq = wk.tile([P, NCT, tc_sz], bf16, tag="xsq")
                    for ct in range(NCT):
                        nc.scalar.activation(xsq[:, ct, :], x2_ch[:, ct, sl],
                                             AF.Square)
                    mps = pp.tile([P, 2, 512], f32, tag="stat")
                    m_b = mps[:, 0, :tc_sz]
                    ss_b = mps[:, 1, :tc_sz]
                    for ct in range(NCT):
                        nc.tensor.matmul(m_b, ones_m[:], x2_ch[:, ct, sl],
                                         start=(ct == 0), stop=(ct == NCT - 1))
                    for ct in range(NCT):
                        nc.tensor.matmul(ss_b, ones_m[:], xsq[:, ct, :],
                                         start=(ct == 0), stop=(ct == NCT - 1))
                    m2 = wk.tile([P, tc_sz], f32, tag="m2")
                    nc.scalar.activation(m2[:], m_b, AF.Square)
                    var = wk.tile([P, tc_sz], f32, tag="var")
                    nc.vector.tensor_tensor(var[:], ss_b, m2[:], ALU.subtract)
                    nc.scalar.activation(s2_img[:, sl], var[:],
                                         AF.Abs_reciprocal_sqrt, bias=zerob[:],
                                         scale=s2_scale)
                    nc.vector.tensor_tensor(t2_img[:, sl], s2_img[:, sl], m_b,
                                            ALU.mult)

                # ---- GEMM phase per chunk ----
                for ch in range(n_chunks):
                    sl = slice(ch * tc_sz, (ch + 1) * tc_sz)
                    # rhs = x2*S2' - T2'  (in place on x2 chunk)
                    for ct in range(NCT):
                        nc.vector.scalar_tensor_tensor(
                            x2_ch[:, ct, sl], x2_ch[:, ct, sl], 0.0,
                            s2_img[:, sl], ALU.bypass, ALU.mult)
                        nc.vector.scalar_tensor_tensor(
                            x2_ch[:, ct, sl], x2_ch[:, ct, sl], 0.0,
                            t2_img[:, sl], ALU.bypass, ALU.subtract)
                    # GEMM1 + gelu + square + q2, two waves of 4 h-tiles
                    hid = wk.tile([P, NHT, tc_sz], bf16, tag="hid")
                    q2 = pp.tile([P, 512], f32, tag="out2", bufs=2)
                    for wv in range(2):
                        sim = pp.tile([P, 4, 512], f32, tag="sim")
                        for k in range(4):
                            ht = wv * 4 + k
                            for ct in range(NCT):
                                nc.tensor.matmul(
                                    sim[:, k, :tc_sz],
                                    w_in_sb[:, ct, ht * P:(ht + 1) * P],
                                    x2_ch[:, ct, sl],
                                    start=(ct == 0), stop=(ct == NCT - 1))
                        nc.scalar.activation(
                            hid[:, wv * 4:(wv + 1) * 4, :],
                            sim[:, :, :tc_sz], AF.Gelu)
                        hsq = wk.tile([P, 4, tc_sz], bf16, tag="hsq")
                        nc.scalar.activation(
                            hsq[:], hid[:, wv * 4:(wv + 1) * 4, :], AF.Square)
                        for k in range(4):
                            nc.tensor.matmul(
                                q2[:, :tc_sz], ones_q[:], hsq[:, k, :],
                                start=(wv == 0 and k == 0),
                                stop=(wv == 1 and k == 3))
                    qinv = wk.tile([P, tc_sz], bf16, tag="qinv")
                    nc.scalar.activation(qinv[:], q2[:, :tc_sz],
                                         AF.Abs_reciprocal_sqrt, bias=zerob[:])
                    # GEMM2 + e2
                    e2_ch = wk.tile([P, NCT, tc_sz], bf16, tag="e2_ch")
                    for ct in range(NCT):
                        o2 = pp.tile([P, 512], f32, tag="out2", bufs=2)
                        for ht in range(NHT):
                            nc.tensor.matmul(
                                o2[:, :tc_sz],
                                w_out_sb[:, ht, ct * P:(ct + 1) * P],
                                hid[:, ht, :],
                                start=(ht == 0), stop=(ht == NHT - 1))
                        nc.vector.scalar_tensor_tensor(
                            e2_ch[:, ct, :], o2[:, :tc_sz],
                            gs_sb[:, ct:ct + 1], qinv[:], ALU.mult, ALU.mult)
                    # e = e1 + e2 (channel layout), one transpose, y = x + e
                    for ct in range(NCT):
                        nc.vector.tensor_tensor(
                            e2_ch[:, ct, :], e2_ch[:, ct, :], e1_ch[:, ct, sl],
                            ALU.add)
                    e_tok = wk.tile([P, ntt_chunk, DIM], bf16, tag="e_tok")
                    for ct in range(NCT):
                        nc.scalar.dma_start_transpose(
                            e_tok[:, :, ct * P:(ct + 1) * P], e2_ch[:, ct, :])
                    xftk = wk.tile([P, ntt_chunk, DIM], f32, tag="xftk")
                    g0 = tok0 + ch * tc_sz
                    nc.sync.dma_start(
                        xftk[:],
                        x_d[g0:g0 + tc_sz, :].rearrange("(a p) b -> p a b", p=P))
                    nc.vector.tensor_tensor(xftk[:], xftk[:], e_tok[:],
                                            ALU.add)
                    nc.sync.dma_start(
                        y_d[g0:g0 + tc_sz, :].rearrange("(a p) b -> p a b", p=P),
                        xftk[:])

    nc.compile()
    return nc


def _prep_params(norm1_w, norm1_b, alpha, norm2_w, norm2_b,
                 proto_in, proto_out, scale_in, scale_out, gamma, t_img):
    import ml_dtypes
    assert np.allclose(norm1_w, 1.0) and np.allclose(norm1_b, 0.0)
    assert np.allclose(norm2_w, 1.0) and np.allclose(norm2_b, 0.0)
    w_in_n = proto_in / np.maximum(
        np.sqrt((proto_in ** 2).sum(0, keepdims=True)), 1e-12)
    w_out_n = proto_out / np.maximum(
        np.sqrt((proto_out ** 2).sum(0, keepdims=True)), 1e-12)
    si = float(np.asarray(scale_in).reshape(-1)[0])
    so = float(np.asarray(scale_out).reshape(-1)[0])
    al = np.asarray(alpha).reshape(-1).astype(np.float32)
    gam = np.asarray(gamma).reshape(-1).astype(np.float32)
    c0 = si / np.sqrt(DIM)
    s2_scale = float(1.0 / (c0 * c0))
    ga = (gam * al).reshape(NCT, P).astype(np.float32)
    gb = (gam * (1.0 - al) / t_img).reshape(NCT, P).astype(np.float32)
    gs = (gam * so).reshape(NCT, P).astype(np.float32)
    w_in_bf = np.ascontiguousarray(
        w_in_n.reshape(NCT, P, HID)).astype(ml_dtypes.bfloat16)
    w_out_bf = np.ascontiguousarray(
        w_out_n.reshape(NHT, P, DIM)).astype(ml_dtypes.bfloat16)
    return w_in_bf, w_out_bf, ga, gb, gs, s2_scale


_NC_CACHE = {}


def kernel(x, norm1_w, norm1_b, alpha, norm2_w, norm2_b,
           proto_in, proto_out, scale_in, scale_out, gamma):
    x = np.asarray(x, dtype=np.float32)
    B, H, W, C = x.shape
    assert C == DIM and B % NCORES == 0
    b_local = B // NCORES
    t_img = H * W

    w_in_bf, w_out_bf, ga, gb, gs, s2_scale = _prep_params(
        np.asarray(norm1_w), np.asarray(norm1_b), np.asarray(alpha),
        np.asarray(norm2_w), np.asarray(norm2_b),
        np.asarray(proto_in, np.float32), np.asarray(proto_out, np.float32),
        np.asarray(scale_in), np.asarray(scale_out), np.asarray(gamma), t_img)

    key = (b_local, H, W, round(s2_scale, 9))
    if key not in _NC_CACHE:
        _NC_CACHE[key] = build_nc(b_local, H, W, s2_scale)
    nc = _NC_CACHE[key]

    xf = x.reshape(B, t_img, DIM)
    in_maps = []
    for core in range(NCORES):
        shard = xf[core * b_local:(core + 1) * b_local].reshape(
            b_local * t_img, DIM)
        in_maps.append({
            "x": np.ascontiguousarray(shard),
            "w_in": w_in_bf, "w_out": w_out_bf,
            "ga": ga, "gb": gb, "gs": gs,
        })
    res = bass_utils.run_bass_kernel_spmd(nc, in_maps,
                                          core_ids=list(range(NCORES)))
    y = np.concatenate([res.results[c]["y"] for c in range(NCORES)], axis=0)
    return y.reshape(B, H, W, C).astype(np.float32)


# revision 18
# speedup vs baseline: 53.4565x; 53.4565x over previous
"""Trainium2 Bass kernel for nn_DeltaBlock: LN -> spatial edge conv -> residual
-> LN -> l2norm'd 2-layer MLP (gelu) -> residual.

Sharding: data-parallel over batch, 16 images / 8 cores = 2 images per core.
All heavy branch compute in bf16 channel-partition layout; the fp32 residual
path stays in token layout (branch outputs are scaled by gamma=1e-4, so bf16
there contributes ~4e-7 relative error on the output).

Math folds (exact, given norm2_w=1/norm2_b=0 which setup_inputs fixes):
  - l2norm(proto_in/out) columns normalized on host (bf16 weights).
  - channel-stage row l2norm of LN2's output: ||LN2(x2)|| = sqrt(DIM*var2)/
    sqrt(var2+eps) * ... folded exactly into the per-token scale
    S2' = (scale_in/sqrt(DIM)) * rsqrt(var2) applied to (x2 - m2).
  - gamma*scale_out folded into the per-channel e2 scale; gamma*alpha and
    gamma*(1-alpha)/HW folded into the e1 activation scale/bias.
"""
import numpy as np
import concourse.bass as bass
import concourse.bacc as bacc
import concourse.mybir as mybir
import concourse.tile as tile
from concourse import bass_utils

f32 = mybir.dt.float32
bf16 = mybir.dt.bfloat16
AF = mybir.ActivationFunctionType
ALU = mybir.AluOpType

DIM = 512
HID = 1024
EPS_LN = 1e-5
NCORES = 8
P = 128
NCT = DIM // P          # 4 channel tiles
NHT = HID // P          # 8 hidden tiles


def build_nc(b_local, h, w, s2_scale, num_devices=NCORES, reps=1, stage=99):
    t_img = h * w
    n_chunks = max(1, t_img // 512)
    tc_sz = t_img // n_chunks
    assert tc_sz <= 512 and tc_sz % P == 0
    ntt_img = t_img // P
    ntt_chunk = tc_sz // P
    hp_h, hp_w = h + 2, w + 2

    nc = bacc.Bacc("TRN2", debug=False, num_devices=num_devices)

    x_d = nc.dram_tensor("x", [b_local * t_img, DIM], f32, kind="ExternalInput")
    w_in_d = nc.dram_tensor("w_in", [NCT, P, HID], bf16, kind="ExternalInput")
    w_out_d = nc.dram_tensor("w_out", [NHT, P, DIM], bf16, kind="ExternalInput")
    ga_d = nc.dram_tensor("ga", [NCT, P], f32, kind="ExternalInput")
    gb_d = nc.dram_tensor("gb", [NCT, P], f32, kind="ExternalInput")
    gs_d = nc.dram_tensor("gs", [NCT, P], f32, kind="ExternalInput")
    y_d = nc.dram_tensor("y", [b_local * t_img, DIM], f32, kind="ExternalOutput")

    with tile.TileContext(nc) as tc:
        with (
            tc.tile_pool(name="const", bufs=1) as cpool,
            tc.tile_pool(name="big", bufs=1) as big,
            tc.tile_pool(name="wk", bufs=2) as wk,
            tc.tile_pool(name="psum", bufs=1, space="PSUM") as pp,
        ):
            # ---- weights / constants ----
            w_in_sb = cpool.tile([P, NCT, HID], bf16, tag="w_in")
            for ct in range(NCT):
                nc.sync.dma_start(w_in_sb[:, ct, :], w_in_d[ct])
            w_out_sb = cpool.tile([P, NHT, DIM], bf16, tag="w_out")
            for ht in range(NHT):
                nc.sync.dma_start(w_out_sb[:, ht, :], w_out_d[ht])
            ga_sb = cpool.tile([P, NCT], f32, tag="ga")
            gb_sb = cpool.tile([P, NCT], f32, tag="gb")
            gs_sb = cpool.tile([P, NCT], f32, tag="gs")
            for v_sb, v_d in ((ga_sb, ga_d), (gb_sb, gb_d), (gs_sb, gs_d)):
                for ct in range(NCT):
                    nc.sync.dma_start(v_sb[:, ct:ct + 1],
                                      v_d[ct].rearrange("(p o) -> p o", o=1))
            ones_m = cpool.tile([P, P], bf16, tag="ones_m")
            nc.vector.memset(ones_m[:], 1.0 / DIM)
            ones_q = cpool.tile([P, P], bf16, tag="ones_q")
            nc.vector.memset(ones_q[:], 1.0)
            epsb = cpool.tile([P, 1], f32, tag="epsb")
            nc.vector.memset(epsb[:], EPS_LN)
            zerob = cpool.tile([P, 1], f32, tag="zerob")
            nc.vector.memset(zerob[:], 0.0)

            for img in range(b_local * reps):
                img = img % b_local
                tok0 = img * t_img

                # ---- x -> bf16 -> channel layout ----
                x_ch = big.tile([P, NCT, t_img], bf16, tag="x_ch")
                for tq in range(ntt_img // 2):
                    xft = wk.tile([P, 2, DIM], f32, tag="xft")
                    g0 = tok0 + tq * 2 * P
                    nc.sync.dma_start(
                        xft[:],
                        x_d[g0:g0 + 2 * P, :].rearrange("(a p) b -> p a b", p=P))
                    for k in range(2):
                        tt = tq * 2 + k
                        xbt = wk.tile([P, DIM], bf16, tag="xbt")
                        nc.scalar.copy(xbt[:], xft[:, k, :])
                        nc.scalar.dma_start_transpose(
                            x_ch[:, :, tt * P:(tt + 1) * P], xbt[:])

                # ---- LN1 stats ----
                if stage < 1:
                    for ch in range(n_chunks):
                        sl = slice(ch * tc_sz, (ch + 1) * tc_sz)
                        xftk = wk.tile([P, ntt_chunk, DIM], f32, tag="xftk", bufs=1)
                        g0 = tok0 + ch * tc_sz
                        nc.sync.dma_start(
                            xftk[:],
                            x_d[g0:g0 + tc_sz, :].rearrange("(a p) b -> p a b", p=P))
                        nc.sync.dma_start(
                            y_d[g0:g0 + tc_sz, :].rearrange("(a p) b -> p a b", p=P),
                            xftk[:])
                    continue
                s1_img = big.tile([P, t_img], bf16, tag="s1_img")
                t1_img = big.tile([P, t_img], bf16, tag="t1_img")
                for ch in range(n_chunks):
                    sl = slice(ch * tc_sz, (ch + 1) * tc_sz)
                    xsq = wk.tile([P, NCT, tc_sz], bf16, tag="xsq", bufs=1)
                    for ct in range(NCT):
                        nc.scalar.activation(xsq[:, ct, :], x_ch[:, ct, sl],
                                             AF.Square)
                    mps = pp.tile([P, 2, 512], f32, tag="stat")
                    m_b = mps[:, 0, :tc_sz]
                    ss_b = mps[:, 1, :tc_sz]
                    for ct in range(NCT):
                        nc.tensor.matmul(m_b, ones_m[:], x_ch[:, ct, sl],
                                         start=(ct == 0), stop=(ct == NCT - 1))
                    for ct in range(NCT):
                        nc.tensor.matmul(ss_b, ones_m[:], xsq[:, ct, :],
                                         start=(ct == 0), stop=(ct == NCT - 1))
                    m2 = wk.tile([P, tc_sz], f32, tag="m2", bufs=1)
                    nc.scalar.activation(m2[:], m_b, AF.Square)
                    var = wk.tile([P, tc_sz], f32, tag="var", bufs=1)
                    nc.vector.tensor_tensor(var[:], ss_b, m2[:], ALU.subtract)
                    nc.scalar.activation(s1_img[:, sl], var[:],
                                         AF.Abs_reciprocal_sqrt, bias=epsb[:])
                    nc.vector.tensor_tensor(t1_img[:, sl], s1_img[:, sl], m_b,
                                            ALU.mult)

                # ---- LN1 apply + spatial + e1 + x2, per channel tile ----
                if stage < 2:
                    continue
                e1_ch = big.tile([P, NCT, t_img], bf16, tag="e1_ch")
                x2_ch = x_ch
                s1_v = s1_img.rearrange("p (a b) -> p a b", b=w)
                t1_v = t1_img.rearrange("p (a b) -> p a b", b=w)
                for ct in range(NCT):
                    # padded h tile: zero borders, write interior = x*s - t
                    hp = wk.tile([P, hp_h, hp_w], bf16, tag="hp", bufs=2)
                    nc.vector.memset(hp[:, 0, :], 0.0)
                    nc.vector.memset(hp[:, hp_h - 1, :], 0.0)
                    nc.vector.memset(hp[:, 1:hp_h - 1, 0:1], 0.0)
                    nc.vector.memset(hp[:, 1:hp_h - 1, hp_w - 1:hp_w], 0.0)
                    gp_acc = wk.tile([P, 1], f32, tag="gp_acc", bufs=2)
                    x_v = x_ch[:, ct, :].rearrange("p (a b) -> p a b", b=w)
                    hin = hp[:, 1:h + 1, 1:w + 1]
                    nc.vector.scalar_tensor_tensor(
                        hin, x_v, 0.0, s1_v, ALU.bypass, ALU.mult)
                    nc.vector.scalar_tensor_tensor(
                        hin, hin, 0.0, t1_v, ALU.bypass, ALU.subtract,
                        accum_out=gp_acc[:])
                    e1b = wk.tile([P, 1], f32, tag="e1b", bufs=2)
                    nc.vector.tensor_tensor(e1b[:], gp_acc[:],
                                            gb_sb[:, ct:ct + 1], ALU.mult)
                    # vertical diffs |dv| over padded rows; feat = |dv|r + |dv|r+1
                    hpf = hp.rearrange("p a b -> p (a b)")
                    dv = wk.tile([P, (hp_h - 1) * hp_w], bf16, tag="dv", bufs=2)
                    feat = wk.tile([P, h, hp_w], bf16, tag="feat", bufs=2)
                    nc.vector.tensor_tensor(
                        dv[:], hpf[:, :(hp_h - 1) * hp_w], hpf[:, hp_w:],
                        ALU.subtract)
                    nc.vector.scalar_tensor_tensor(
                        dv[:], dv[:], -1.0, dv[:], ALU.mult, ALU.max)
                    featf = feat.rearrange("p a b -> p (a b)")
                    nc.vector.tensor_tensor(
                        featf[:], dv[:, :h * hp_w], dv[:, hp_w:], ALU.add)
                    # horizontal diffs on interior rows
                    dh = wk.tile([P, h, hp_w - 1], bf16, tag="dv", bufs=2)
                    nc.vector.tensor_tensor(
                        dh[:], hp[:, 1:h + 1, :hp_w - 1], hp[:, 1:h + 1, 1:],
                        ALU.subtract)
                    nc.vector.scalar_tensor_tensor(
                        dh[:], dh[:], -1.0, dh[:], ALU.mult, ALU.max)
                    nc.vector.tensor_tensor(
                        feat[:, :, 1:], feat[:, :, 1:], dh[:], ALU.add)
                    nc.vector.tensor_tensor(
                        feat[:, :, :hp_w - 1], feat[:, :, :hp_w - 1], dh[:],
                        ALU.add)
                    # e1 = ga*feat + gb*gp ; x2 = x + e1
                    nc.scalar.activation(
                        e1_ch[:, ct, :].rearrange("p (a b) -> p a b", b=w),
                        feat[:, :, 1:w + 1], AF.Identity,
                        bias=e1b[:], scale=ga_sb[:, ct:ct + 1])
                    nc.gpsimd.tensor_tensor(
                        x2_ch[:, ct, :], x_ch[:, ct, :], e1_ch[:, ct, :],
                        ALU.add)

                # ---- LN2 stats: S2' = Abs_rsqrt(var2*s2_scale), T2' = S2'*m2 ----
                if stage < 3:
                    continue
                s2_img = s1_img
                t2_img = t1_img
                for ch in range(n_chunks):
                    sl = slice(ch * tc_sz, (ch + 1) * tc_sz)
                    xsq = wk.tile([P, NCT, tc_sz], bf16, tag="xsq", bufs=1)
                    for ct in range(NCT):
                        nc.scalar.activation(xsq[:, ct, :], x2_ch[:, ct, sl],
                                             AF.Square)
                    mps = pp.tile([P, 2, 512], f32, tag="stat")
                    m_b = mps[:, 0, :tc_sz]
                    ss_b = mps[:, 1, :tc_sz]
                    for ct in range(NCT):
                        nc.tensor.matmul(m_b, ones_m[:], x2_ch[:, ct, sl],
                                         start=(ct == 0), stop=(ct == NCT - 1))
                    for ct in range(NCT):
                        nc.tensor.matmul(ss_b, ones_m[:], xsq[:, ct, :],
                                         start=(ct == 0), stop=(ct == NCT - 1))
                    m2 = wk.tile([P, tc_sz], f32, tag="m2", bufs=1)
                    nc.scalar.activation(m2[:], m_b, AF.Square)
                    var = wk.tile([P, tc_sz], f32, tag="var", bufs=1)
                    nc.vector.tensor_tensor(var[:], ss_b, m2[:], ALU.subtract)
                    nc.scalar.activation(s2_img[:, sl], var[:],
                                         AF.Abs_reciprocal_sqrt, bias=zerob[:],
                                         scale=s2_scale)
                    nc.vector.tensor_tensor(t2_img[:, sl], s2_img[:, sl], m_b,
                                            ALU.mult)

                # ---- GEMM phase per chunk ----
                if stage < 4:
                    continue
                for ch in range(n_chunks):
                    sl = slice(ch * tc_sz, (ch + 1) * tc_sz)
                    # rhs = x2*S2' - T2'  (in place on x2 chunk)
                    for ct in range(NCT):
                        nc.vector.scalar_tensor_tensor(
                            x2_ch[:, ct, sl], x2_ch[:, ct, sl], 0.0,
                            s2_img[:, sl], ALU.bypass, ALU.mult)
                        nc.vector.scalar_tensor_tensor(
                            x2_ch[:, ct, sl], x2_ch[:, ct, sl], 0.0,
                            t2_img[:, sl], ALU.bypass, ALU.subtract)
                    # GEMM1 + gelu + square + q2, two waves of 4 h-tiles
                    hid = wk.tile([P, NHT, tc_sz], bf16, tag="hid", bufs=1)
                    q2 = pp.tile([P, 512], f32, tag="out2", bufs=2)
                    for wv in range(2):
                        sim = pp.tile([P, 4, 512], f32, tag="sim")
                        for k in range(4):
                            ht = wv * 4 + k
                            for ct in range(NCT):
                                nc.tensor.matmul(
                                    sim[:, k, :tc_sz],
                                    w_in_sb[:, ct, ht * P:(ht + 1) * P],
                                    x2_ch[:, ct, sl],
                                    start=(ct == 0), stop=(ct == NCT - 1))
                        nc.scalar.activation(
                            hid[:, wv * 4:(wv + 1) * 4, :],
                            sim[:, :, :tc_sz], AF.Gelu)
                        hsq = wk.tile([P, 4, tc_sz], bf16, tag="hsq")
                        nc.scalar.activation(
                            hsq[:], hid[:, wv * 4:(wv + 1) * 4, :], AF.Square)
                        for k in range(4):
                            nc.tensor.matmul(
                                q2[:, :tc_sz], ones_q[:], hsq[:, k, :],
                                start=(wv == 0 and k == 0),
                                stop=(wv == 1 and k == 3))
                    qinv = wk.tile([P, tc_sz], bf16, tag="qinv")
                    nc.scalar.activation(qinv[:], q2[:, :tc_sz],
                                         AF.Abs_reciprocal_sqrt, bias=zerob[:])
                    # GEMM2 + e2
                    e2_ch = wk.tile([P, NCT, tc_sz], bf16, tag="e2_ch")
                    for ct in range(NCT):
                        o2 = pp.tile([P, 512], f32, tag="out2", bufs=2)
                        for ht in range(NHT):
                            nc.tensor.matmul(
                                o2[:, :tc_sz],
                                w_out_sb[:, ht, ct * P:(ct + 1) * P],
                                hid[:, ht, :],
                                start=(ht == 0), stop=(ht == NHT - 1))
                        nc.vector.scalar_tensor_tensor(
                            e2_ch[:, ct, :], o2[:, :tc_sz],
                            gs_sb[:, ct:ct + 1], qinv[:], ALU.mult, ALU.mult)
                    # e = e1 + e2 (channel layout), one transpose, y = x + e
                    for ct in range(NCT):
                        nc.vector.tensor_tensor(
                            e2_ch[:, ct, :], e2_ch[:, ct, :], e1_ch[:, ct, sl],
                            ALU.add)
                    e_tok = wk.tile([P, ntt_chunk, DIM], bf16, tag="e_tok")
                    for ct in range(NCT):
                        nc.scalar.dma_start_transpose(
                            e_tok[:, :, ct * P:(ct + 1) * P], e2_ch[:, ct, :])
                    xftk = wk.tile([P, ntt_chunk, DIM], f32, tag="xftk", bufs=1)
                    g0 = tok0 + ch * tc_sz
                    nc.sync.dma_start(
                        xftk[:],
                        x_d[g0:g0 + tc_sz, :].rearrange("(a p) b -> p a b", p=P))
                    nc.gpsimd.tensor_tensor(xftk[:], xftk[:], e_tok[:],
                                            ALU.add)
                    nc.sync.dma_start(
                        y_d[g0:g0 + tc_sz, :].rearrange("(a p) b -> p a b", p=P),
                        xftk[:])

    nc.compile()
    return nc


def _prep_params(norm1_w, norm1_b, alpha, norm2_w, norm2_b,
                 proto_in, proto_out, scale_in, scale_out, gamma, t_img):
    import ml_dtypes
    assert np.allclose(norm1_w, 1.0) and np.allclose(norm1_b, 0.0)
    assert np.allclose(norm2_w, 1.0) and np.allclose(norm2_b, 0.0)
    w_in_n = proto_in / np.maximum(
        np.sqrt((proto_in ** 2).sum(0, keepdims=True)), 1e-12)
    w_out_n = proto_out / np.maximum(
        np.sqrt((proto_out ** 2).sum(0, keepdims=True)), 1e-12)
    si = float(np.asarray(scale_in).reshape(-1)[0])
    so = float(np.asarray(scale_out).reshape(-1)[0])
    al = np.asarray(alpha).reshape(-1).astype(np.float32)
    gam = np.asarray(gamma).reshape(-1).astype(np.float32)
    c0 = si / np.sqrt(DIM)
    s2_scale = float(1.0 / (c0 * c0))
    ga = (gam * al).reshape(NCT, P).astype(np.float32)
    gb = (gam * (1.0 - al) / t_img).reshape(NCT, P).astype(np.float32)
    gs = (gam * so).reshape(NCT, P).astype(np.float32)
    w_in_bf = np.ascontiguousarray(
        w_in_n.reshape(NCT, P, HID)).astype(ml_dtypes.bfloat16)
    w_out_bf = np.ascontiguousarray(
        w_out_n.reshape(NHT, P, DIM)).astype(ml_dtypes.bfloat16)
    return w_in_bf, w_out_bf, ga, gb, gs, s2_scale


_NC_CACHE = {}


def kernel(x, norm1_w, norm1_b, alpha, norm2_w, norm2_b,
           proto_in, proto_out, scale_in, scale_out, gamma):
    x = np.asarray(x, dtype=np.float32)
    B, H, W, C = x.shape
    assert C == DIM and B % NCORES == 0
    b_local = B // NCORES
    t_img = H * W

    w_in_bf, w_out_bf, ga, gb, gs, s2_scale = _prep_params(
        np.asarray(norm1_w), np.asarray(norm1_b), np.asarray(alpha),
        np.asarray(norm2_w), np.asarray(norm2_b),
        np.asarray(proto_in, np.float32), np.asarray(proto_out, np.float32),
        np.asarray(scale_in), np.asarray(scale_out), np.asarray(gamma), t_img)

    key = (b_local, H, W, round(s2_scale, 9))
    if key not in _NC_CACHE:
        _NC_CACHE[key] = build_nc(b_local, H, W, s2_scale)
    nc = _NC_CACHE[key]

    xf = x.reshape(B, t_img, DIM)
    in_maps = []
    for core in range(NCORES):
        shard = xf[core * b_local:(core + 1) * b_local].reshape(
            b_local * t_img, DIM)
        in_maps.append({
            "x": np.ascontiguousarray(shard),
            "w_in": w_in_bf, "w_out": w_out_bf,
            "ga": ga, "gb": gb, "gs": gs,
        })
    res = bass_utils.run_bass_kernel_spmd(nc, in_maps,
                                          core_ids=list(range(NCORES)))
    y = np.concatenate([res.results[c]["y"] for c in range(NCORES)], axis=0)
    return y.reshape(B, H, W, C).astype(np.float32)
